# revision 47
# baseline (speedup 1.0000x reference)
"""Trainium2 Bass kernel for the per-expert masked-MLP problem.

Computation (reference):
    x[b,i,d] = inputs[b,d] * adjacency[i,d]
    h1 = relu(einsum('bid,idh->bih', x, W1) + b1)
    h2 = relu(einsum('bih,ihk->bik', h1, W2) + b2)
    out[b,i] = einsum('bih,ih->bi', h2, W3) + b3

Shapes: B=4096, D=128 (experts == input dim), H=256.

Strategy: expert-parallel across 8 NeuronCores (16 experts per core),
fp16 matmuls, transposed dataflow (layer outputs [h, b] / [k, b]).

Redesign vs the 255us baseline (which spent 2 of 8 matmuls per
expert-tile streaming h2 through the PE with one-hot W3 lhsT columns).
Measured ~212us on HW (scale-relative error ~2.5e-3, gate 2e-2).
Known-regressing variants: DEFER=4 (noise-worse), h1/h2 bufs 6/8/8,
splitting the first w1 DMA, sgns on the sync queue (+3.4us), b2all
before xT[512:1024] (+2us), WARM=14 (+3us), DVE dma_start (invalid).

  * |W3| is folded into W2/b2 on the HOST (free): the z2 drains produce
    h2'[k,b] = |W3[k]| * relu(z2[k,b] + b2[k]) >= 0 with a plain
    relu+bias op.  The SIGN of W3 moves into the L3 lhsT (+-1 entries).
  * the k axis (256) is permuted per-expert on the host so partitions k
    and k+128 always carry the SAME W3 sign (sign of the min-|W3[k]|
    element is flipped when the positive count is odd: ~1e-4 error).
    The idle GpSimd engine then PRE-SUMS h2'a + h2'b into one
    [128, 512] tile, and layer 3 needs ONE matmul per expert-tile
    (lhsT = +-1 step column) instead of two: 7 PE matmuls per
    expert-tile instead of 8 (-27us PE).
  * quad q's l3 bank accumulates its 4 experts at rows 32q..32q+3
    (engine ops need 32-aligned start partitions), so a single [4,512]
    ACT drain with b3 bias finishes the output - no cross-quad adds.
  * L3 flushes deferred DEFER=3 quad-tiles; drained one-per-slot over
    the final two tiles; the FINAL tile skips the presum and feeds
    h2'a/h2'b straight into 8 L3 matmuls (keeps the serialized gpsimd
    chain off the tail; hybrid per-expert split measured worse).
    Last quad DMAs out per-tile ([4,512] 8KB).
  * z1 emission runs one expert-slot AHEAD of z2 emission (software
    pipelining), so the PE never waits in-order on the h1 drain of the
    expert it just computed.
  * sgn/b3 lhsT tiles ship pre-built from the host (no on-device
    scatter); input DMAs split across sync (weights) and scalar
    (x tiles + b1) queues as in the baseline.
"""

import numpy as np

import concourse.tile as tile
from concourse import bacc, mybir
from concourse import bass_utils

B = 4096
D = 128
H = 256
NCORES = 8
NE = D // NCORES  # experts per core = 16

F32 = mybir.dt.float32
F16 = mybir.dt.float16

BT = 512  # batch tile width (PSUM bank = 512 fp32)
NBT = B // BT  # 8 batch tiles

CW = 12 * H  # per-quad chunk width in wts: [w1 (4H) | w2a (4H) | w2b (4H)]

WARM = 5  # warm-up matmuls: bridge PE to first-data-ready (~10us)
DEFER = 3  # L3 deferral depth in quad-tiles


def _w1_base(e):
    return (e // 4) * CW + (e % 4) * H


def _w2a_base(e):
    return (e // 4) * CW + 4 * H + (e % 4) * H


def _w2b_base(e):
    return (e // 4) * CW + 8 * H + (e % 4) * H


def _emit(tc: tile.TileContext, outs, ins):
    from contextlib import ExitStack

    ctx = ExitStack()
    nc = tc.nc
    xinT = ins["xinT"]  # [128, B] f16   (inputs, host-transposed)
    wts = ins["wts"]  # [128, 3*NE*H] f16  (W1 pre-masked, W2' = W2*|W3| perm)
    sgns = ins["sgns"]  # [128, NE*128] f16 (expert e: col 32(e//4)+e%4 = +-1)
    b1ab = ins["b1ab"]  # [128, 2*NE] f32 (b1 halves side by side)
    b2all = ins["b2all"]  # [128, 2*NE+1] f32 (b2' halves | b3 scattered)
    outT = outs["outT"]  # [NE, B] f32

    consts = ctx.enter_context(tc.tile_pool(name="consts", bufs=1))

    xT = consts.tile([128, B], F16, name="xT")
    b1ab_sb = consts.tile([128, 2 * NE], F32, name="b1ab_sb")
    b2all_sb = consts.tile([128, 2 * NE + 1], F32, name="b2all_sb")
    sgns_sb = consts.tile([128, NE * 128], F16, name="sgns_sb")
    wts_sb = consts.tile([128, 3 * NE * H], F16, name="wts_sb")
    warm_src = consts.tile([128, 128 + BT], F16, name="warm_src")
    nc.vector.memset(warm_src, 0.0)

    # ---- DMA order = critical path.  scalar queue: only the transfers
    # the first few drains need (ACT must be free for z1 drains early).
    nc.scalar.dma_start(out=xT[:, 0:512], in_=xinT[:, 0:512])
    nc.scalar.dma_start(out=b1ab_sb, in_=b1ab)
    nc.scalar.dma_start(out=xT[:, 512:1024], in_=xinT[:, 512:1024])
    nc.scalar.dma_start(out=b2all_sb, in_=b2all)
    nc.scalar.dma_start(out=sgns_sb, in_=sgns)
    # sync queue: weights in consumption order, split so the completion
    # semaphore for each piece fires as early as possible.
    nc.sync.dma_start(out=wts_sb[:, 0:512], in_=wts[:, 0:512])  # w1 e0,e1
    nc.sync.dma_start(
        out=wts_sb[:, 4 * H : 4 * H + 512], in_=wts[:, 4 * H : 4 * H + 512]
    )  # w2a e0,e1
    nc.sync.dma_start(
        out=wts_sb[:, 8 * H : 8 * H + 512], in_=wts[:, 8 * H : 8 * H + 512]
    )  # w2b e0,e1
    nc.sync.dma_start(out=wts_sb[:, 512:1024], in_=wts[:, 512:1024])  # w1 e2,e3
    nc.sync.dma_start(
        out=wts_sb[:, 4 * H + 512 : 8 * H], in_=wts[:, 4 * H + 512 : 8 * H]
    )
    nc.sync.dma_start(
        out=wts_sb[:, 8 * H + 512 : 12 * H], in_=wts[:, 8 * H + 512 : 12 * H]
    )
    nc.sync.dma_start(out=xT[:, 1024:2048], in_=xinT[:, 1024:2048])
    nc.sync.dma_start(out=xT[:, 2048:3072], in_=xinT[:, 2048:3072])
    nc.sync.dma_start(out=xT[:, 3072:], in_=xinT[:, 3072:])
    for c in range(1, 4):
        nc.sync.dma_start(
            out=wts_sb[:, c * CW : c * CW + 4 * H],
            in_=wts[:, c * CW : c * CW + 4 * H],
        )
        nc.sync.dma_start(
            out=wts_sb[:, c * CW + 4 * H : (c + 1) * CW],
            in_=wts[:, c * CW + 4 * H : (c + 1) * CW],
        )

    outT_sb = consts.tile([128, B], F32, name="outT_sb")

    # ---- HAM warm-up: keep the PE busy (clock ungated) while the input
    # DMAs stream in.  Full-width lhsT so each runs at the 213ns floor.
    warm_sink = consts.tile([128, 1], F32, name="warm_sink")
    with tc.tile_pool(name="warmpool", bufs=1, space="PSUM") as warmpool:
        warm = warmpool.tile([128, BT], F32, name="warm", tag="warm")
        for _ in range(WARM):
            nc.tensor.matmul(
                warm, warm_src[:, 0:128], warm_src[:, 128 : 128 + BT],
                start=True, stop=True,
            )
        nc.vector.tensor_copy(out=warm_sink, in_=warm[:, 0:1])

    # ---- main loop ---------------------------------------------------
    hpool = ctx.enter_context(tc.tile_pool(name="hpool", bufs=6))
    zpool = ctx.enter_context(tc.tile_pool(name="zpool", bufs=7, space="PSUM"))
    l3pool = ctx.enter_context(tc.tile_pool(name="l3pool", bufs=1, space="PSUM"))

    relu = mybir.ActivationFunctionType.Relu
    ident = mybir.ActivationFunctionType.Identity

    def emit_z1(e, bsl):
        z1a = zpool.tile([128, BT], F32, name="z1a", tag="z")
        z1b = zpool.tile([128, BT], F32, name="z1b", tag="z")
        wb = _w1_base(e)
        nc.tensor.matmul(
            z1a, wts_sb[:, wb : wb + 128], xT[:, bsl], start=True, stop=True
        )
        nc.tensor.matmul(
            z1b, wts_sb[:, wb + 128 : wb + H], xT[:, bsl], start=True, stop=True
        )
        h1 = hpool.tile([128, 2 * BT], F16, name="h1", tag="h1", bufs=5)
        nc.scalar.activation(
            out=h1[:, 0:BT], in_=z1a, func=relu, bias=b1ab_sb[:, e : e + 1]
        )
        nc.scalar.activation(
            out=h1[:, BT : 2 * BT],
            in_=z1b,
            func=relu,
            bias=b1ab_sb[:, NE + e : NE + e + 1],
        )
        return h1

    def emit_z2(e, h1, skip_ps=False):
        z2a = zpool.tile([128, BT], F32, name="z2a", tag="z")
        z2b = zpool.tile([128, BT], F32, name="z2b", tag="z")
        for kh, z2t in ((0, z2a), (1, z2b)):
            ba = _w2a_base(e) + kh * 128
            bb = _w2b_base(e) + kh * 128
            nc.tensor.matmul(
                z2t, wts_sb[:, ba : ba + 128], h1[:, 0:BT], start=True, stop=False
            )
            nc.tensor.matmul(
                z2t,
                wts_sb[:, bb : bb + 128],
                h1[:, BT : 2 * BT],
                start=False,
                stop=True,
            )
        h2a = hpool.tile([128, BT], F16, name="h2a", tag="h2a", bufs=6)
        h2b = hpool.tile([128, BT], F16, name="h2b", tag="h2b", bufs=6)
        nc.vector.tensor_scalar(
            out=h2a,
            in0=z2a,
            scalar1=b2all_sb[:, e : e + 1],
            scalar2=0.0,
            op0=mybir.AluOpType.add,
            op1=mybir.AluOpType.max,
        )
        nc.vector.tensor_scalar(
            out=h2b,
            in0=z2b,
            scalar1=b2all_sb[:, NE + e : NE + e + 1],
            scalar2=0.0,
            op0=mybir.AluOpType.add,
            op1=mybir.AluOpType.max,
        )
        if skip_ps:
            # final quad-tile: L3 consumes h2a/h2b directly (the A/B
            # halves share the sign vector), keeping the serialized
            # gpsimd presum chain off the kernel tail.
            return (h2a, h2b)
        ps = hpool.tile([128, BT], F16, name="ps", tag="ps", bufs=26)
        nc.gpsimd.tensor_tensor(out=ps, in0=h2a, in1=h2b, op=mybir.AluOpType.add)
        return ps

    drained = [0, 0, 0, 0]

    def emit_l3(q, bsl, pss):
        l3 = l3pool.tile([128, BT], F32, name="l3", tag="l3")
        rhss = []
        for i, ps in enumerate(pss):
            sg = sgns_sb[:, (4 * q + i) * 128 : (4 * q + i + 1) * 128]
            if isinstance(ps, tuple):
                rhss.extend((sg, r) for r in ps)
            else:
                rhss.append((sg, ps))
        for k, (sg, rhs) in enumerate(rhss):
            nc.tensor.matmul(
                l3, sg, rhs, start=(k == 0), stop=(k == len(rhss) - 1)
            )
        # engine ops need 32-aligned start partitions: quad q lives at
        # l3/outT_sb partitions 32q..32q+3 (lhsT sign columns 32q+i)
        nc.scalar.activation(
            out=outT_sb[32 * q : 32 * q + 4, bsl],
            in_=l3[32 * q : 32 * q + 4, :],
            func=ident,
            bias=b2all_sb[32 * q : 32 * q + 4, 2 * NE : 2 * NE + 1],
        )
        drained[q] += 1
        if q == 3:
            # last quad: per-tile 8KB DMAs keep the final transfer tiny
            nc.sync.dma_start(out=outT[12:16, bsl], in_=outT_sb[96:100, bsl])
        elif drained[q] == NBT:
            nc.sync.dma_start(
                out=outT[4 * q : 4 * q + 4, :], in_=outT_sb[32 * q : 32 * q + 4, :]
            )

    slots = [(q, t, i) for q in range(4) for t in range(NBT) for i in range(4)]
    flushed = [0]
    pend = []  # [(q, bsl, [ps0..ps3])]
    group = []  # presums of the current quad-tile
    prev = None  # (q, t, i, bsl, h1)
    for q, t, i in slots:
        bsl = slice(t * BT, (t + 1) * BT)
        h1 = emit_z1(4 * q + i, bsl)
        if prev is not None:
            pq, pt, pi, pbsl, ph1 = prev
            # drain the pend backlog gradually over the final two tiles
            endgame = pq == 3 and (
                (pt == NBT - 2 and len(pend) > 1) or pt == NBT - 1
            )
            # the first two flushes wait one quad-tile longer: the
            # gpsimd presum pipeline is still filling at ~17us and the
            # first flush otherwise waits ~0.5us on its 4th presum
            depth = DEFER + (1 if flushed[0] < 2 else 0)
            if (pi == 2 and len(pend) > depth) or (endgame and pend):
                fq, fbsl, fps = pend.pop(0)
                emit_l3(fq, fbsl, fps)
                flushed[0] += 1
            last_tile = pq == 3 and pt == NBT - 1
            group.append((pq, pbsl, emit_z2(4 * pq + pi, ph1, last_tile)))
            if pi == 3:
                pend.append((pq, group[0][1], [g[2] for g in group]))
                group = []
        prev = (q, t, i, bsl, h1)
    pq, pt, pi, pbsl, ph1 = prev
    group.append((pq, pbsl, emit_z2(4 * pq + pi, ph1, True)))
    pend.append((pq, group[0][1], [g[2] for g in group]))
    for fq, fbsl, fps in pend:
        emit_l3(fq, fbsl, fps)

    ctx.close()


def build_nc():
    nc = bacc.Bacc("TRN2", target_bir_lowering=False, debug=False)
    ins = {
        "xinT": nc.dram_tensor("xinT", [128, B], F16, kind="ExternalInput").ap(),
        "wts": nc.dram_tensor(
            "wts", [128, 3 * NE * H], F16, kind="ExternalInput"
        ).ap(),
        "sgns": nc.dram_tensor(
            "sgns", [128, NE * 128], F16, kind="ExternalInput"
        ).ap(),
        "b1ab": nc.dram_tensor(
            "b1ab", [128, 2 * NE], F32, kind="ExternalInput"
        ).ap(),
        "b2all": nc.dram_tensor(
            "b2all", [128, 2 * NE + 1], F32, kind="ExternalInput"
        ).ap(),
    }
    outs = {
        "outT": nc.dram_tensor("outT", [NE, B], F32, kind="ExternalOutput").ap(),
    }
    with tile.TileContext(nc) as tc:
        _emit(tc, outs, ins)
    nc.compile()
    return nc


def make_in_maps(inputs, adjacency, W1, b1, W2, b2, W3, b3):
    xinT = np.ascontiguousarray(
        np.asarray(inputs, dtype=np.float32).T.astype(np.float16)
    )
    adjacency = np.asarray(adjacency, dtype=np.float32)
    W2f = np.asarray(W2, dtype=np.float32)
    W3f = np.asarray(W3, dtype=np.float32)
    b2f = np.asarray(b2, dtype=np.float32)
    in_maps = []
    for c in range(NCORES):
        es = slice(c * NE, (c + 1) * NE)
        # fold the adjacency mask into W1 on the host (free)
        w1c = np.asarray(W1[es], dtype=np.float32) * adjacency[es][:, :, None]
        w1d = w1c.transpose(1, 0, 2)  # [128, NE, H]
        # per-expert: fold |W3| into W2/b2, permute k so partitions k and
        # k+128 share the W3 sign, record the +-1 step vector.
        w2p = np.empty((NE, H, H), dtype=np.float32)
        b2p = np.empty((NE, H), dtype=np.float32)
        sgns = np.zeros((128, NE * 128), dtype=np.float16)
        b3sp = np.zeros((128, 1), dtype=np.float32)
        b3v = np.asarray(b3[es], dtype=np.float32)
        for j in range(NE):
            b3sp[32 * (j // 4) + (j % 4), 0] = b3v[j]
        for j in range(NE):
            e = c * NE + j
            w3 = W3f[e].copy()  # [256]
            s = np.where(w3 >= 0, 1.0, -1.0)
            if int((s > 0).sum()) % 2 == 1:
                kstar = int(np.argmin(np.abs(w3)))
                s[kstar] = -s[kstar]
            pos = np.where(s > 0)[0]
            neg = np.where(s < 0)[0]
            a = len(pos) // 2
            Ah = np.concatenate([pos[:a], neg[: 128 - a]])
            Bh = np.concatenate([pos[a:], neg[128 - a :]])
            perm = np.concatenate([Ah, Bh]).astype(np.int64)
            w3a = np.abs(w3)
            w2p[j] = W2f[e][:, perm] * w3a[perm][None, :]
            b2p[j] = (b2f[e] * w3a)[perm]
            sgn = np.where(np.arange(128) < a, 1.0, -1.0)
            # quad q = j//4 lands at l3 rows 32q..32q+3 (32-aligned
            # start-partition requirement for the ACT drain)
            sgns[:, j * 128 + 32 * (j // 4) + (j % 4)] = sgn.astype(np.float16)
        w2ad = w2p[:, 0:128, :].transpose(1, 0, 2)  # [128(h), NE, H(k)]
        w2bd = w2p[:, 128:256, :].transpose(1, 0, 2)
        wts = np.empty((128, 3 * NE * H), dtype=np.float16)
        for g in range(4):
            es4 = slice(g * 4, (g + 1) * 4)
            wts[:, g * CW : g * CW + 4 * H] = w1d[:, es4].reshape(128, 4 * H)
            wts[:, g * CW + 4 * H : g * CW + 8 * H] = w2ad[:, es4].reshape(
                128, 4 * H
            )
            wts[:, g * CW + 8 * H : g * CW + 12 * H] = w2bd[:, es4].reshape(
                128, 4 * H
            )
        b1es = np.asarray(b1[es], dtype=np.float32)  # [NE, 256]
        b1ab = np.empty((128, 2 * NE), dtype=np.float32)
        b1ab[:, 0:NE] = b1es[:, 0:128].T
        b1ab[:, NE : 2 * NE] = b1es[:, 128:256].T
        b2all = np.empty((128, 2 * NE + 1), dtype=np.float32)
        b2all[:, 0:NE] = b2p[:, 0:128].T
        b2all[:, NE : 2 * NE] = b2p[:, 128:256].T
        b2all[:, 2 * NE] = b3sp[:, 0]
        in_maps.append(
            {
                "xinT": xinT,
                "wts": wts,
                "sgns": sgns,
                "b1ab": b1ab,
                "b2all": b2all,
            }
        )
    return in_maps


_NC_CACHE = []


def _get_nc():
    if not _NC_CACHE:
        _NC_CACHE.append(build_nc())
    return _NC_CACHE[0]


def run_on_cores(in_maps, trace=False, **kwargs):
    nc = _get_nc()
    return bass_utils.run_bass_kernel_spmd(
        nc, in_maps, core_ids=list(range(NCORES)), trace=trace, **kwargs
    )


def assemble_out(results):
    """results: list of 8 per-core dicts with 'outT' [NE, B]."""
    out = np.empty((B, D), dtype=np.float32)
    for c in range(NCORES):
        out[:, c * NE : (c + 1) * NE] = results[c]["outT"].T
    return out


def kernel(inputs, adjacency, W1, b1, W2, b2, W3, b3):
    in_maps = make_in_maps(inputs, adjacency, W1, b1, W2, b2, W3, b3)
    res = run_on_cores(in_maps, trace=False)
    return assemble_out(res.results)
